# revision 21
# baseline (speedup 1.0000x reference)
"""Multihead causal attention on 8 TRN2 NeuronCores.

Sharding: core = (batch b, head-group hg): b = core//2, hg = core%2.
Each core gets x[b] (full sequence, [2048, 1024]) plus the weight rows for
its 8 heads (W[hg*512:(hg+1)*512, :]), computes Q/K/V projections and
causal attention for those (batch, head) pairs, and writes unnormalized
Y transposed as [8, 65, 2048] (head, dh+den, seq) in bf16; the host
divides by the denominator row and transposes back on gather.

Per-core dataflow:
  - x.T and the weights arrive pre-transposed and pre-tiled by the host as
    [128, 8, *] arrays so each load is ONE wide DMA (the Sync queue issues
    DMA instructions at ~0.6us each; per-chunk DMAs would serialize).
  - All matmuls run in bf16 (fp8 was tested and rejected: quantization
    noise in a dot product does not average down with K, so fp8 inputs put
    ~3%% error on Q/K and the softmax output).
  - Attention is 16 independent (head-pair g, q-chunk qt) blocks. Block
    order puts the smallest blocks last; projection chunks are scheduled
    as PE filler by earliest-deadline-first so the PE covers each block's
    exp-throughput deficit.
  - Scores in transposed layout scoresT[k, q] = K @ Q.T per head, two
    heads per 128-row group (K=64 each, concurrent via PE row tiling).
    One [128, 2, 512] PSUM tile per k-tile holds both heads' scores.
  - Software pipeline inside a block: scores run TWO k-tiles ahead of PV
    (psS double buffer + deep ee pool), so PV's exp dependency is always
    already satisfied and the PE never exposes its drain latency waiting
    on the Scalar engine.
  - Causal masking at 128-column granularity: diagonal 128x128 blocks get
    a triangular 0/1 multiply on DVE after exp.
  - Softmax without a max pass; exp mostly on ScalarE (PSUM -> bf16 SBUF,
    one instruction per k-tile covering both heads). A subset of k-tiles
    in the Scalar-paced blocks computes exp on DVE instead, via a
    Schraudolph bit-trick (int16(x*a+b) whose bits are the bf16 result).
  - A burst of tiny warmup matmuls runs during the initial DMA wait so the
    PE's HAM clock gate is already at full rate when real work starts.
"""
import numpy as np
import ml_dtypes

import concourse.bass as bass
import concourse.tile as tile
from concourse import bacc, mybir
from concourse.bass_utils import run_bass_kernel_spmd

F32 = mybir.dt.float32
BF16 = mybir.dt.bfloat16
I16 = mybir.dt.int16
EXP = mybir.ActivationFunctionType.Exp
MULT = mybir.AluOpType.mult
ADD = mybir.AluOpType.add

B, S, D, H, DH = 4, 2048, 1024, 16, 64
N_CORES = 8
H_LOC = 8          # heads per core
D_LOC = H_LOC * DH  # 512: projection output dim per core
N_CT = D // 128     # 8 contraction tiles
N_ST = S // 128     # 16 sequence tiles of 128
N_QT = S // 512     # 4 q-chunks of 512
W_SCALE = 1.0
SCALE = 1.0 / np.sqrt(DH)

# Schraudolph fast-exp constants for exp(x*SCALE) via int16 bitcast to bf16:
#   bf16_bits = int16(x * EXP_A + EXP_B)   (truncating convert)
EXP_A = 128.0 * SCALE / np.log(2.0)
EXP_B = 16249.0

# blocks (g, qt); order chosen so the tail blocks have the smallest
# exp-vs-PE deficit (qt=0) and late Q chunks remain available as filler.
BLOCK_ORDER = [(0, 0), (0, 1), (0, 2), (0, 3),
               (1, 1), (1, 2), (1, 3),
               (2, 1), (2, 2), (2, 3),
               (3, 1), (3, 2), (3, 3),
               (1, 0), (2, 0), (3, 0)]


def _dve_kts(pos, qt):
    """k-tiles whose exp runs on DVE (Schraudolph) instead of ScalarE,
    chosen where the Scalar engine paces the block. Blocks at the end of
    the schedule (pos >= 10) have no projection filler left, so they get
    a larger share."""
    late = pos >= 10
    if qt == 1:
        return frozenset({1, 4} if late else {1})
    if qt == 2:
        return frozenset({1, 3, 5, 7, 9} if late else {1, 4, 7})
    if qt == 3:
        return frozenset({1, 3, 5, 7, 9, 11} if late else {1, 4, 7, 10})
    return frozenset()


_NC_CACHE = {}


def _deficit_ns(pos, qt):
    """Per-block exp-throughput deficit the PE must cover with filler."""
    total = 0.0
    n_kt = 4 * (qt + 1)
    dve = _dve_kts(pos, qt)
    for kt in range(n_kt):
        off = max(0, (kt - 4 * qt) * 128)
        cols = 512 - off
        exp_ns = (2 * cols * 0.833 + 280.0) if kt not in dve else 0.0
        pe_ns = 3 * cols / 2.4
        total += max(0.0, exp_ns - pe_ns)
    return total


def build_nc():
    nc = bacc.Bacc("TRN2", target_bir_lowering=False, debug=False,
                   num_devices=N_CORES)
    # host-pretiled inputs: [128, ct, *] so one DMA covers all 8 ct chunks
    xtd = nc.dram_tensor("xtd", [128, N_CT, S], BF16,
                         kind="ExternalInput").ap()
    wqt = nc.dram_tensor("wqt", [128, N_CT, D_LOC], BF16,
                         kind="ExternalInput").ap()
    wkt = nc.dram_tensor("wkt", [128, N_CT, D_LOC], BF16,
                         kind="ExternalInput").ap()
    wvt = nc.dram_tensor("wvt", [128, N_CT, D_LOC], BF16,
                         kind="ExternalInput").ap()
    out = nc.dram_tensor("out", [H_LOC, DH + 1, S], BF16,
                         kind="ExternalOutput").ap()

    # tri[kk, qq] = 1 iff qq >= kk, duplicated side by side so one multiply
    # masks both heads' planes
    tri_np = (np.arange(128)[None, :] >= np.arange(128)[:, None])
    tri2_np = np.concatenate([tri_np, tri_np], axis=1)
    tri_dram = nc.inline_tensor(tri2_np.astype(ml_dtypes.bfloat16),
                                name="tri")

    with tile.TileContext(nc) as tc:
        with tc.tile_pool(name="consts", bufs=1) as consts, \
             tc.tile_pool(name="pers", bufs=1) as pers, \
             tc.tile_pool(name="xw", bufs=1) as xw, \
             tc.tile_pool(name="epool", bufs=8) as epool, \
             tc.tile_pool(name="ypool", bufs=2) as ypool, \
             tc.tile_pool(name="psP", bufs=2, space="PSUM") as psP, \
             tc.tile_pool(name="psS", bufs=2, space="PSUM") as psS, \
             tc.tile_pool(name="psY", bufs=1, space="PSUM") as psY:
            tri = consts.tile([128, 2, 128], BF16)
            nc.sync.dma_start(tri[:], tri_dram.ap())

            # input tiles (persistent): x.T and the three weights, each a
            # single wide tile [128, ct, cols]
            xT = xw.tile([128, N_CT, S], BF16, tag="xT", name="xT")
            WK = xw.tile([128, N_CT, D_LOC], BF16, tag="WK", name="WK")
            WQ = xw.tile([128, N_CT, D_LOC], BF16, tag="WQ", name="WQ")
            WV = xw.tile([128, N_CT, D_LOC], BF16, tag="WV", name="WV")

            # DMA priority: what block (0,0) needs first (K/Q g-0 column
            # slices + x q-chunk 0 + full WV for V projections), then the
            # rest in rough consumption order. ~11 wide DMAs total.
            nc.sync.dma_start(WK[:, 0:4, 0:128], wkt[:, 0:4, 0:128])
            nc.sync.dma_start(xT[:, 0:4, 0:512], xtd[:, 0:4, 0:512])
            nc.sync.dma_start(WK[:, 4:8, 0:128], wkt[:, 4:8, 0:128])
            nc.sync.dma_start(xT[:, 4:8, 0:512], xtd[:, 4:8, 0:512])
            nc.sync.dma_start(WQ[:, :, 0:128], wqt[:, :, 0:128])
            nc.sync.dma_start(WV[:, 0:4, :], wvt[:, 0:4, :])
            nc.sync.dma_start(WV[:, 4:8, :], wvt[:, 4:8, :])
            nc.sync.dma_start(xT[:, 0:4, 512:1024], xtd[:, 0:4, 512:1024])
            nc.sync.dma_start(xT[:, 4:8, 512:1024], xtd[:, 4:8, 512:1024])
            nc.sync.dma_start(WK[:, :, 128:D_LOC], wkt[:, :, 128:D_LOC])
            nc.sync.dma_start(WQ[:, :, 128:D_LOC], wqt[:, :, 128:D_LOC])
            nc.sync.dma_start(xT[:, :, 1024:1536], xtd[:, :, 1024:1536])
            nc.sync.dma_start(xT[:, :, 1536:2048], xtd[:, :, 1536:2048])

            # HAM warmup: tiny matmuls on a locally-memset tile (no DMA
            # dependency, so they start as soon as the engines boot) keep the
            # PE busy while the input DMAs land and open the HAM clock gate
            # before real work starts.
            wsrc = consts.tile([128, 128], BF16, tag="wsrc", name="wsrc")
            nc.gpsimd.memset(wsrc[:], 0.5)
            warm = psS.tile([128, 2, 512], F32, tag="s", name="warm")
            for _ in range(45):
                nc.tensor.matmul(warm[:, 0, 0:128], wsrc[:],
                                 wsrc[:], start=True, stop=True,
                                 skip_group_check=True)

            # persistent per-core tensors
            QT = [pers.tile([128, S], BF16, tag=f"QT{i}", name=f"QT{i}")
                  for i in range(4)]
            KT = [pers.tile([128, S], BF16, tag=f"KT{i}", name=f"KT{i}")
                  for i in range(4)]
            VP = [pers.tile([128, H_LOC, DH + 1], BF16, tag=f"VP{i}",
                            name=f"VP{i}") for i in range(N_ST)]
            # "ones" column of V: W_SCALE so the denominator matches the
            # host-scaled V rows (gpsimd, off the critical path)
            for st in range(N_ST):
                nc.gpsimd.memset(VP[st][:, :, DH:DH + 1], W_SCALE)

            def proj_qk(dst, W, g, qc, part, st8, copy_eng):
                # dst[g][:, qc*512:(qc+1)*512] = W[:, g-block].T @ x.T chunk
                # fp8 DoubleRow: each matmul contracts ct pair (2j, 2j+1)
                if part == 0:
                    st8["pp"] = psP.tile([128, 512], F32, tag="pp",
                                         name="pp")
                pp = st8["pp"]
                cts = range(4) if part == 0 else range(4, N_CT)
                for ct in cts:
                    nc.tensor.matmul(
                        pp[:],
                        W[:, ct, g * 128:(g + 1) * 128],
                        xT[:, ct, qc * 512:(qc + 1) * 512],
                        start=(ct == 0), stop=(ct == N_CT - 1),
                        skip_group_check=True)
                if part != 0:
                    copy_eng(dst[g][:, qc * 512:(qc + 1) * 512], pp[:])

            def proj_v(st, part, st8, copy_eng):
                if part == 0:
                    st8["pp"] = psP.tile([128, 512], F32, tag="pp",
                                         name="pp")
                pp = st8["pp"]
                cts = range(4) if part == 0 else range(4, N_CT)
                for ct in cts:
                    nc.tensor.matmul(
                        pp[:],
                        xT[:, ct, st * 128:(st + 1) * 128],
                        WV[:, ct, :],
                        start=(ct == 0), stop=(ct == N_CT - 1),
                        skip_group_check=True)
                if part != 0:
                    copy_eng(VP[st][:, :, 0:DH],
                             pp[:].rearrange("p (h d) -> p h d", h=H_LOC))

            def copy_dve(dst, src):
                nc.vector.tensor_copy(dst, src)

            def copy_act(dst, src):
                nc.scalar.copy(dst, src)

            # ---- filler chunk bookkeeping (earliest-deadline-first) ----
            first_block_at = {}  # (g, qt) -> position
            for i, (g, qt) in enumerate(BLOCK_ORDER):
                first_block_at[(g, qt)] = i

            def first_use_kq(g, qc):
                return min(i for i, (gg, qq) in enumerate(BLOCK_ORDER)
                           if gg == g and qq >= qc)

            def first_use_v(st):
                return min(i for i, (gg, qq) in enumerate(BLOCK_ORDER)
                           if 4 * (qq + 1) > st)

            chunks = []  # [deadline, seq, kind, cost_ns, make_thunks]
            seq = 0
            for g in range(4):
                for qc in range(N_QT):
                    # K(g,qc) feeds every (g, qt>=qc) block; Q(g,qc) only
                    # its own block. Both must land a block early.
                    for W, nm, dl in ((KT, WK, first_use_kq(g, qc) - 1),
                                      (QT, WQ, first_block_at[(g, qc)] - 1)):
                        def mk(W=W, nm=nm, g=g, qc=qc):
                            st8 = {}
                            return [
                                lambda ce: proj_qk(W, nm, g, qc, 0, st8, ce),
                                lambda ce: proj_qk(W, nm, g, qc, 1, st8, ce)]
                        chunks.append([dl, seq, "kq", 1720.0, mk])
                        seq += 1
            for st in range(N_ST):
                def mkv(st=st):
                    st8 = {}
                    return [lambda ce: proj_v(st, 0, st8, ce),
                            lambda ce: proj_v(st, 1, st8, ce)]
                chunks.append([first_use_v(st), seq, ("v", st), 1720.0, mkv])
                seq += 1
            chunks.sort(key=lambda c: (c[0], c[1]))

            def take_chunks(pos, budget_ns):
                """Pop mandatory chunks (deadline <= pos) plus EDF-optional
                ones until budget is covered. Returns (v_chunks, others)."""
                got_v, got_o, got_cost = [], [], 0.0
                rest = []
                for c in chunks:
                    dl, _, kind, cost, mk = c
                    if dl <= pos or got_cost < budget_ns:
                        if isinstance(kind, tuple):  # v chunk
                            got_v.append((kind[1], mk()))
                        else:
                            got_o.append(mk())
                        got_cost += cost
                    else:
                        rest.append(c)
                chunks[:] = rest
                return got_v, got_o

            # ---- pipeline fill: block 0's K/Q ----
            for c in list(chunks):
                if c[0] < 0:
                    for t in c[4]():
                        t(copy_dve)
                    chunks.remove(c)

            for pos, (g, qt) in enumerate(BLOCK_ORDER):
                n_kt = 4 * (qt + 1)
                q0 = qt * 512
                dve_kts = _dve_kts(pos, qt)
                # early blocks have Scalar-engine slack: let it do the
                # projection drains there, keeping DVE free
                ce = copy_act if pos <= 2 else copy_dve
                yy = [psY.tile([DH + 1, 512], F32, tag=f"y{hh}",
                               name=f"y{hh}") for hh in range(2)]

                v_units, other_units = take_chunks(pos, _deficit_ns(pos, qt))
                # slots keyed by pair-iteration: v units pinned no later
                # than the iteration before their PV; others spread evenly
                n_it = n_kt // 2 + 1
                slots = {}
                for st, thunks in v_units:
                    kt_pin = st if 4 * qt <= st < n_kt else 0
                    slots.setdefault(max(0, kt_pin // 2), []).extend(thunks)
                for i, u in enumerate(other_units):
                    slots.setdefault(i * n_it // max(1, len(other_units)),
                                     []).extend(u)

                def scores(kt):
                    off = max(0, (kt - 4 * qt) * 128)
                    ps2 = psS.tile([128, 2, 512], F32, tag="s", name="s")
                    for hh in range(2):
                        rows = slice(hh * 64, hh * 64 + 64)
                        nc.tensor.matmul(
                            ps2[:, hh, off:512],
                            KT[g][rows, kt * 128:(kt + 1) * 128],
                            QT[g][rows, q0 + off:q0 + 512],
                            start=True, stop=True)
                    return ps2

                def exp_tile(kt, ps2):
                    off = max(0, (kt - 4 * qt) * 128)
                    ee = epool.tile([128, 2, 512], BF16, tag="e", name="e")
                    if kt in dve_kts:
                        # Schraudolph exp on DVE: bf16 bits via int16 cast
                        nc.vector.tensor_scalar(
                            ee[:, :, off:512].bitcast(I16),
                            ps2[:, :, off:512], EXP_A, EXP_B, MULT, ADD)
                    else:
                        nc.scalar.activation(ee[:, :, off:512],
                                             ps2[:, :, off:512], EXP,
                                             scale=SCALE)
                    if kt >= 4 * qt:  # diagonal block: causal mask
                        nc.vector.tensor_mul(
                            ee[:, :, off:off + 128],
                            ee[:, :, off:off + 128],
                            tri[:])
                    return ee

                def pv(kt, ee):
                    off = max(0, (kt - 4 * qt) * 128)
                    for hh in range(2):
                        nc.tensor.matmul(
                            yy[hh][:, off:512],
                            VP[kt][:, 2 * g + hh, :],
                            ee[:, hh, off:512],
                            start=(kt == 0), stop=(kt == n_kt - 1),
                            skip_group_check=True)

                # software pipeline over k-tile PAIRS: both score pairs of
                # a k-tile pair are emitted adjacently (tiled-mode LDWEIGHTS
                # overlap tiled-mode matmuls, but not full-mode ones, so
                # batching halves the mode-transition stalls); PV trails by
                # two pair-iterations so its exp input is always long done.
                # Output columns j*128..(j+1)*128 are final after pv(4qt+j),
                # so the bulk of the drain (cols 0:384) runs early and only
                # the last 128 columns gate the block boundary (psY reuse).
                yn = ypool.tile([DH + 1, 2, 512], BF16, tag="yn", name="yn")
                ps = [scores(0), scores(1)]
                es = [exp_tile(0, ps[0]), exp_tile(1, ps[1])]
                n_pair = n_kt // 2
                for i in range(n_pair + 1):
                    if i >= 1:
                        for kt in (2 * i - 2, 2 * i - 1):
                            pv(kt, es[kt])
                            if kt == 4 * qt + 2:
                                # late blocks carry the big DVE exp share;
                                # put their bulk drain on ScalarE instead
                                bulk = (nc.scalar.copy if pos >= 10
                                        else nc.vector.tensor_copy)
                                for hh in range(2):
                                    bulk(yn[:, hh, 0:384], yy[hh][:, 0:384])
                    for thunk in slots.get(i, ()):
                        thunk(ce)
                    if 2 * i + 2 < n_kt:
                        ps.append(scores(2 * i + 2))
                        ps.append(scores(2 * i + 3))
                        es.append(exp_tile(2 * i + 2, ps[2 * i + 2]))
                        es.append(exp_tile(2 * i + 3, ps[2 * i + 3]))
                for hh in range(2):
                    # ScalarE: its queue is empty at block boundaries, so
                    # the psY-gating final drain is not stuck behind DVE work
                    nc.scalar.copy(yn[:, hh, 384:512],
                                   yy[hh][:, 384:512])
                nc.sync.dma_start(
                    out[2 * g:2 * g + 2, :, q0:q0 + 512]
                    .rearrange("h p c -> p h c"),
                    yn[:])
    nc.compile()
    return nc


def get_nc():
    if "nc" not in _NC_CACHE:
        _NC_CACHE["nc"] = build_nc()
    return _NC_CACHE["nc"]


def make_in_maps(x, W_q, W_k, W_v):
    bf = ml_dtypes.bfloat16

    def tile8(a, scale):  # [1024, cols] -> [128, 8, cols]
        return np.ascontiguousarray(
            (a * scale).reshape(N_CT, 128, -1).transpose(1, 0, 2)
            .astype(bf))

    in_maps = []
    for core in range(N_CORES):
        b, hg = core // 2, core % 2
        rows = slice(hg * D_LOC, (hg + 1) * D_LOC)
        in_maps.append({
            "xtd": tile8(np.asarray(x[b], dtype=np.float32).T, 1.0),
            "wqt": tile8(np.asarray(W_q[rows], dtype=np.float32).T, W_SCALE),
            "wkt": tile8(np.asarray(W_k[rows], dtype=np.float32).T, W_SCALE),
            "wvt": tile8(np.asarray(W_v[rows], dtype=np.float32).T, W_SCALE),
        })
    return in_maps


def assemble(results):
    Y = np.empty((B, H, S, DH), dtype=np.float32)
    for core in range(N_CORES):
        b, hg = core // 2, core % 2
        yc = np.asarray(results[core]["out"], dtype=np.float32)  # [H_LOC, DH+1, S]
        yn = yc[:, 0:DH, :] / yc[:, DH:DH + 1, :]
        Y[b, hg * H_LOC:(hg + 1) * H_LOC] = yn.transpose(0, 2, 1)
    return Y


def kernel(x, W_q, W_k, W_v):
    nc = get_nc()
    in_maps = make_in_maps(x, W_q, W_k, W_v)
    res = run_bass_kernel_spmd(nc, in_maps, list(range(N_CORES)))
    return assemble(res.results)


# revision 22
# speedup vs baseline: 1.0269x; 1.0269x over previous
"""Multihead causal attention on 8 TRN2 NeuronCores.

Sharding: core = (batch b, head-group hg): b = core//2, hg = core%2.
Each core gets x[b] (full sequence, [2048, 1024]) plus the weight rows for
its 8 heads (W[hg*512:(hg+1)*512, :]), computes Q/K/V projections and
causal attention for those (batch, head) pairs, and writes unnormalized
Y transposed as [8, 65, 2048] (head, dh+den, seq) in bf16; the host
divides by the denominator row and transposes back on gather.

Per-core dataflow:
  - x.T and the weights arrive pre-transposed and pre-tiled by the host as
    [128, 8, *] arrays so each load is ONE wide DMA (the Sync queue issues
    DMA instructions at ~0.6us each; per-chunk DMAs would serialize).
  - All matmuls run in bf16 (fp8 was tested and rejected: quantization
    noise in a dot product does not average down with K, so fp8 inputs put
    ~3%% error on Q/K and the softmax output).
  - Attention is 16 independent (head-pair g, q-chunk qt) blocks. Block
    order puts the smallest blocks last; projection chunks are scheduled
    as PE filler by earliest-deadline-first so the PE covers each block's
    exp-throughput deficit.
  - Scores in transposed layout scoresT[k, q] = K @ Q.T per head, two
    heads per 128-row group (K=64 each, concurrent via PE row tiling).
    One [128, 2, 512] PSUM tile per k-tile holds both heads' scores.
  - Software pipeline inside a block: scores run TWO k-tiles ahead of PV
    (psS double buffer + deep ee pool), so PV's exp dependency is always
    already satisfied and the PE never exposes its drain latency waiting
    on the Scalar engine.
  - Causal masking at 128-column granularity: diagonal 128x128 blocks get
    a triangular 0/1 multiply on DVE after exp.
  - Softmax without a max pass; exp mostly on ScalarE (PSUM -> bf16 SBUF,
    one instruction per k-tile covering both heads). A subset of k-tiles
    in the Scalar-paced blocks computes exp on DVE instead, via a
    Schraudolph bit-trick (int16(x*a+b) whose bits are the bf16 result).
  - A burst of tiny warmup matmuls runs during the initial DMA wait so the
    PE's HAM clock gate is already at full rate when real work starts.
"""
import numpy as np
import ml_dtypes

import concourse.bass as bass
import concourse.tile as tile
from concourse import bacc, mybir
from concourse.bass_utils import run_bass_kernel_spmd

F32 = mybir.dt.float32
BF16 = mybir.dt.bfloat16
I16 = mybir.dt.int16
EXP = mybir.ActivationFunctionType.Exp
MULT = mybir.AluOpType.mult
ADD = mybir.AluOpType.add

B, S, D, H, DH = 4, 2048, 1024, 16, 64
N_CORES = 8
H_LOC = 8          # heads per core
D_LOC = H_LOC * DH  # 512: projection output dim per core
N_CT = D // 128     # 8 contraction tiles
N_ST = S // 128     # 16 sequence tiles of 128
N_QT = S // 512     # 4 q-chunks of 512
W_SCALE = 1.0
SCALE = 1.0 / np.sqrt(DH)

# Schraudolph fast-exp constants for exp(x*SCALE) via int16 bitcast to bf16:
#   bf16_bits = int16(x * EXP_A + EXP_B)   (truncating convert)
EXP_A = 128.0 * SCALE / np.log(2.0)
EXP_B = 16249.0

# blocks (g, qt); order chosen so the tail blocks have the smallest
# exp-vs-PE deficit (qt=0) and late Q chunks remain available as filler.
BLOCK_ORDER = [(0, 0), (0, 1), (0, 2), (0, 3),
               (1, 1), (1, 2), (1, 3),
               (2, 1), (2, 2), (2, 3),
               (3, 1), (3, 2), (3, 3),
               (1, 0), (2, 0), (3, 0)]


def _dve_kts(pos, qt):
    """k-tiles whose exp runs on DVE (Schraudolph) instead of ScalarE,
    chosen where the Scalar engine paces the block. Blocks at the end of
    the schedule (pos >= 10) have no projection filler left, so they get
    a larger share."""
    late = pos >= 10
    if qt == 1:
        return frozenset({1, 4} if late else {1})
    if qt == 2:
        return frozenset({1, 3, 5, 7, 9} if late else {1, 4, 7})
    if qt == 3:
        return frozenset({1, 3, 5, 7, 9, 11} if late else {1, 4, 7, 10})
    return frozenset()


_NC_CACHE = {}


def _deficit_ns(pos, qt):
    """Per-block exp-throughput deficit the PE must cover with filler."""
    total = 0.0
    n_kt = 4 * (qt + 1)
    dve = _dve_kts(pos, qt)
    for kt in range(n_kt):
        off = max(0, (kt - 4 * qt) * 128)
        cols = 512 - off
        exp_ns = (2 * cols * 0.833 + 280.0) if kt not in dve else 0.0
        pe_ns = 3 * cols / 2.4
        total += max(0.0, exp_ns - pe_ns)
    return total


def build_nc():
    nc = bacc.Bacc("TRN2", target_bir_lowering=False, debug=False,
                   num_devices=N_CORES)
    # host-pretiled inputs: [128, ct, *] so one DMA covers all 8 ct chunks
    xtd = nc.dram_tensor("xtd", [128, N_CT, S], BF16,
                         kind="ExternalInput").ap()
    wqt = nc.dram_tensor("wqt", [128, N_CT, D_LOC], BF16,
                         kind="ExternalInput").ap()
    wkt = nc.dram_tensor("wkt", [128, N_CT, D_LOC], BF16,
                         kind="ExternalInput").ap()
    wvt = nc.dram_tensor("wvt", [128, N_CT, D_LOC], BF16,
                         kind="ExternalInput").ap()
    out = nc.dram_tensor("out", [H_LOC, DH + 1, S], BF16,
                         kind="ExternalOutput").ap()

    # tri[kk, qq] = 1 iff qq >= kk, duplicated side by side so one multiply
    # masks both heads' planes
    tri_np = (np.arange(128)[None, :] >= np.arange(128)[:, None])
    tri2_np = np.concatenate([tri_np, tri_np], axis=1)
    tri_dram = nc.inline_tensor(tri2_np.astype(ml_dtypes.bfloat16),
                                name="tri")

    with tile.TileContext(nc) as tc:
        with tc.tile_pool(name="consts", bufs=1) as consts, \
             tc.tile_pool(name="pers", bufs=1) as pers, \
             tc.tile_pool(name="xw", bufs=1) as xw, \
             tc.tile_pool(name="epool", bufs=8) as epool, \
             tc.tile_pool(name="ypool", bufs=2) as ypool, \
             tc.tile_pool(name="psP", bufs=2, space="PSUM") as psP, \
             tc.tile_pool(name="psS", bufs=2, space="PSUM") as psS, \
             tc.tile_pool(name="psY", bufs=1, space="PSUM") as psY:
            tri = consts.tile([128, 2, 128], BF16)
            nc.sync.dma_start(tri[:], tri_dram.ap())

            # input tiles (persistent): x.T and the three weights, each a
            # single wide tile [128, ct, cols]
            xT = xw.tile([128, N_CT, S], BF16, tag="xT", name="xT")
            WK = xw.tile([128, N_CT, D_LOC], BF16, tag="WK", name="WK")
            WQ = xw.tile([128, N_CT, D_LOC], BF16, tag="WQ", name="WQ")
            WV = xw.tile([128, N_CT, D_LOC], BF16, tag="WV", name="WV")

            # DMA priority: what block (0,0) needs first (K/Q g-0 column
            # slices + x q-chunk 0 + full WV for V projections), then the
            # rest in rough consumption order. ~11 wide DMAs total.
            nc.sync.dma_start(WK[:, 0:4, 0:128], wkt[:, 0:4, 0:128])
            nc.sync.dma_start(xT[:, 0:4, 0:512], xtd[:, 0:4, 0:512])
            nc.sync.dma_start(WK[:, 4:8, 0:128], wkt[:, 4:8, 0:128])
            nc.sync.dma_start(xT[:, 4:8, 0:512], xtd[:, 4:8, 0:512])
            nc.sync.dma_start(WQ[:, :, 0:128], wqt[:, :, 0:128])
            nc.sync.dma_start(WV[:, 0:4, :], wvt[:, 0:4, :])
            nc.sync.dma_start(WV[:, 4:8, :], wvt[:, 4:8, :])
            nc.sync.dma_start(xT[:, 0:4, 512:1024], xtd[:, 0:4, 512:1024])
            nc.sync.dma_start(xT[:, 4:8, 512:1024], xtd[:, 4:8, 512:1024])
            nc.sync.dma_start(WK[:, :, 128:D_LOC], wkt[:, :, 128:D_LOC])
            nc.sync.dma_start(WQ[:, :, 128:D_LOC], wqt[:, :, 128:D_LOC])
            nc.sync.dma_start(xT[:, :, 1024:1536], xtd[:, :, 1024:1536])
            nc.sync.dma_start(xT[:, :, 1536:2048], xtd[:, :, 1536:2048])

            # HAM warmup: tiny matmuls on a locally-memset tile (no DMA
            # dependency, so they start as soon as the engines boot) keep the
            # PE busy while the input DMAs land and open the HAM clock gate
            # before real work starts.
            wsrc = consts.tile([128, 128], BF16, tag="wsrc", name="wsrc")
            nc.gpsimd.memset(wsrc[:], 0.5)
            warm = psS.tile([128, 2, 512], F32, tag="s", name="warm")
            for _ in range(45):
                nc.tensor.matmul(warm[:, 0, 0:128], wsrc[:],
                                 wsrc[:], start=True, stop=True,
                                 skip_group_check=True)

            # persistent per-core tensors
            QT = [pers.tile([128, S], BF16, tag=f"QT{i}", name=f"QT{i}")
                  for i in range(4)]
            KT = [pers.tile([128, S], BF16, tag=f"KT{i}", name=f"KT{i}")
                  for i in range(4)]
            VP = [pers.tile([128, H_LOC, DH + 1], BF16, tag=f"VP{i}",
                            name=f"VP{i}") for i in range(N_ST)]
            # "ones" column of V: W_SCALE so the denominator matches the
            # host-scaled V rows (gpsimd, off the critical path)
            for st in range(N_ST):
                nc.gpsimd.memset(VP[st][:, :, DH:DH + 1], W_SCALE)

            def proj_qk(dst, W, g, qc, part, st8, copy_eng):
                # dst[g][:, qc*512:(qc+1)*512] = W[:, g-block].T @ x.T chunk
                # fp8 DoubleRow: each matmul contracts ct pair (2j, 2j+1)
                if part == 0:
                    st8["pp"] = psP.tile([128, 512], F32, tag="pp",
                                         name="pp")
                pp = st8["pp"]
                cts = range(4) if part == 0 else range(4, N_CT)
                for ct in cts:
                    nc.tensor.matmul(
                        pp[:],
                        W[:, ct, g * 128:(g + 1) * 128],
                        xT[:, ct, qc * 512:(qc + 1) * 512],
                        start=(ct == 0), stop=(ct == N_CT - 1),
                        skip_group_check=True)
                if part != 0:
                    copy_eng(dst[g][:, qc * 512:(qc + 1) * 512], pp[:])

            def proj_v(st, part, st8, copy_eng):
                if part == 0:
                    st8["pp"] = psP.tile([128, 512], F32, tag="pp",
                                         name="pp")
                pp = st8["pp"]
                cts = range(4) if part == 0 else range(4, N_CT)
                for ct in cts:
                    nc.tensor.matmul(
                        pp[:],
                        xT[:, ct, st * 128:(st + 1) * 128],
                        WV[:, ct, :],
                        start=(ct == 0), stop=(ct == N_CT - 1),
                        skip_group_check=True)
                if part != 0:
                    copy_eng(VP[st][:, :, 0:DH],
                             pp[:].rearrange("p (h d) -> p h d", h=H_LOC))

            def copy_dve(dst, src):
                nc.vector.tensor_copy(dst, src)

            def copy_act(dst, src):
                nc.scalar.copy(dst, src)

            # ---- filler chunk bookkeeping (earliest-deadline-first) ----
            first_block_at = {}  # (g, qt) -> position
            for i, (g, qt) in enumerate(BLOCK_ORDER):
                first_block_at[(g, qt)] = i

            def first_use_kq(g, qc):
                return min(i for i, (gg, qq) in enumerate(BLOCK_ORDER)
                           if gg == g and qq >= qc)

            def first_use_v(st):
                return min(i for i, (gg, qq) in enumerate(BLOCK_ORDER)
                           if 4 * (qq + 1) > st)

            chunks = []  # [deadline, seq, kind, cost_ns, make_thunks]
            seq = 0
            for g in range(4):
                for qc in range(N_QT):
                    # K(g,qc) feeds every (g, qt>=qc) block; Q(g,qc) only
                    # its own block. Both must land a block early.
                    for W, nm, dl in ((KT, WK, first_use_kq(g, qc) - 1),
                                      (QT, WQ, first_block_at[(g, qc)] - 1)):
                        def mk(W=W, nm=nm, g=g, qc=qc):
                            st8 = {}
                            return [
                                lambda ce: proj_qk(W, nm, g, qc, 0, st8, ce),
                                lambda ce: proj_qk(W, nm, g, qc, 1, st8, ce)]
                        chunks.append([dl, seq, "kq", 1720.0, mk])
                        seq += 1
            for st in range(N_ST):
                def mkv(st=st):
                    st8 = {}
                    return [lambda ce: proj_v(st, 0, st8, ce),
                            lambda ce: proj_v(st, 1, st8, ce)]
                chunks.append([first_use_v(st), seq, ("v", st), 1720.0, mkv])
                seq += 1
            chunks.sort(key=lambda c: (c[0], c[1]))

            def take_chunks(pos, budget_ns):
                """Pop mandatory chunks (deadline <= pos) plus EDF-optional
                ones until budget is covered. Returns (v_chunks, others)."""
                got_v, got_o, got_cost = [], [], 0.0
                rest = []
                for c in chunks:
                    dl, _, kind, cost, mk = c
                    if dl <= pos or got_cost < budget_ns:
                        if isinstance(kind, tuple):  # v chunk
                            got_v.append((kind[1], mk()))
                        else:
                            got_o.append(mk())
                        got_cost += cost
                    else:
                        rest.append(c)
                chunks[:] = rest
                return got_v, got_o

            # ---- pipeline fill: block 0's K/Q ----
            for c in list(chunks):
                if c[0] < 0:
                    for t in c[4]():
                        t(copy_dve)
                    chunks.remove(c)

            for pos, (g, qt) in enumerate(BLOCK_ORDER):
                n_kt = 4 * (qt + 1)
                q0 = qt * 512
                dve_kts = _dve_kts(pos, qt)
                # early blocks have Scalar-engine slack: let it do the
                # projection drains there, keeping DVE free
                ce = copy_act if pos <= 2 else copy_dve
                yy = [psY.tile([DH + 1, 512], F32, tag=f"y{hh}",
                               name=f"y{hh}") for hh in range(2)]

                v_units, other_units = take_chunks(pos, _deficit_ns(pos, qt))
                # slots keyed by pair-iteration: v units pinned no later
                # than the iteration before their PV; others spread evenly
                n_it = n_kt // 2 + 1
                slots = {}
                for st, thunks in v_units:
                    kt_pin = st if 4 * qt <= st < n_kt else 0
                    slots.setdefault(max(0, kt_pin // 2), []).extend(thunks)
                for i, u in enumerate(other_units):
                    slots.setdefault(i * n_it // max(1, len(other_units)),
                                     []).extend(u)

                def scores(kt):
                    off = max(0, (kt - 4 * qt) * 128)
                    ps2 = psS.tile([128, 2, 512], F32, tag="s", name="s")
                    for hh in range(2):
                        rows = slice(hh * 64, hh * 64 + 64)
                        nc.tensor.matmul(
                            ps2[:, hh, off:512],
                            KT[g][rows, kt * 128:(kt + 1) * 128],
                            QT[g][rows, q0 + off:q0 + 512],
                            start=True, stop=True)
                    return ps2

                def exp_tile(kt, ps2):
                    off = max(0, (kt - 4 * qt) * 128)
                    ee = epool.tile([128, 2, 512], BF16, tag="e", name="e")
                    if kt in dve_kts:
                        # Schraudolph exp on DVE: bf16 bits via int16 cast
                        nc.vector.tensor_scalar(
                            ee[:, :, off:512].bitcast(I16),
                            ps2[:, :, off:512], EXP_A, EXP_B, MULT, ADD)
                    else:
                        nc.scalar.activation(ee[:, :, off:512],
                                             ps2[:, :, off:512], EXP,
                                             scale=SCALE)
                    if kt >= 4 * qt:  # diagonal block: causal mask
                        nc.vector.tensor_mul(
                            ee[:, :, off:off + 128],
                            ee[:, :, off:off + 128],
                            tri[:])
                    return ee

                def pv(kt, ee):
                    off = max(0, (kt - 4 * qt) * 128)
                    for hh in range(2):
                        nc.tensor.matmul(
                            yy[hh][:, off:512],
                            VP[kt][:, 2 * g + hh, :],
                            ee[:, hh, off:512],
                            start=(kt == 0), stop=(kt == n_kt - 1),
                            skip_group_check=True)

                # software pipeline over k-tile PAIRS: both score pairs of
                # a k-tile pair are emitted adjacently (tiled-mode LDWEIGHTS
                # overlap tiled-mode matmuls, but not full-mode ones, so
                # batching halves the mode-transition stalls); PV trails by
                # two pair-iterations so its exp input is always long done.
                # Output columns j*128..(j+1)*128 are final after pv(4qt+j),
                # so the bulk of the drain (cols 0:384) runs early and only
                # the last 128 columns gate the block boundary (psY reuse).
                yn = ypool.tile([DH + 1, 2, 512], BF16, tag="yn", name="yn")
                ps = [scores(0), scores(1)]
                es = [exp_tile(0, ps[0]), exp_tile(1, ps[1])]
                n_pair = n_kt // 2
                for i in range(n_pair + 1):
                    if i >= 1:
                        for kt in (2 * i - 2, 2 * i - 1):
                            pv(kt, es[kt])
                            if kt == 4 * qt + 2:
                                for hh in range(2):
                                    nc.vector.tensor_copy(
                                        yn[:, hh, 0:384], yy[hh][:, 0:384])
                    for thunk in slots.get(i, ()):
                        thunk(ce)
                    if 2 * i + 2 < n_kt:
                        ps.append(scores(2 * i + 2))
                        ps.append(scores(2 * i + 3))
                        es.append(exp_tile(2 * i + 2, ps[2 * i + 2]))
                        es.append(exp_tile(2 * i + 3, ps[2 * i + 3]))
                for hh in range(2):
                    # ScalarE: its queue is empty at block boundaries, so
                    # the psY-gating final drain is not stuck behind DVE work
                    nc.scalar.copy(yn[:, hh, 384:512],
                                   yy[hh][:, 384:512])
                nc.sync.dma_start(
                    out[2 * g:2 * g + 2, :, q0:q0 + 512]
                    .rearrange("h p c -> p h c"),
                    yn[:])
    nc.compile()
    return nc


def get_nc():
    if "nc" not in _NC_CACHE:
        _NC_CACHE["nc"] = build_nc()
    return _NC_CACHE["nc"]


def make_in_maps(x, W_q, W_k, W_v):
    bf = ml_dtypes.bfloat16

    def tile8(a, scale):  # [1024, cols] -> [128, 8, cols]
        return np.ascontiguousarray(
            (a * scale).reshape(N_CT, 128, -1).transpose(1, 0, 2)
            .astype(bf))

    in_maps = []
    for core in range(N_CORES):
        b, hg = core // 2, core % 2
        rows = slice(hg * D_LOC, (hg + 1) * D_LOC)
        in_maps.append({
            "xtd": tile8(np.asarray(x[b], dtype=np.float32).T, 1.0),
            "wqt": tile8(np.asarray(W_q[rows], dtype=np.float32).T, W_SCALE),
            "wkt": tile8(np.asarray(W_k[rows], dtype=np.float32).T, W_SCALE),
            "wvt": tile8(np.asarray(W_v[rows], dtype=np.float32).T, W_SCALE),
        })
    return in_maps


def assemble(results):
    Y = np.empty((B, H, S, DH), dtype=np.float32)
    for core in range(N_CORES):
        b, hg = core // 2, core % 2
        yc = np.asarray(results[core]["out"], dtype=np.float32)  # [H_LOC, DH+1, S]
        yn = yc[:, 0:DH, :] / yc[:, DH:DH + 1, :]
        Y[b, hg * H_LOC:(hg + 1) * H_LOC] = yn.transpose(0, 2, 1)
    return Y


def kernel(x, W_q, W_k, W_v):
    nc = get_nc()
    in_maps = make_in_maps(x, W_q, W_k, W_v)
    res = run_bass_kernel_spmd(nc, in_maps, list(range(N_CORES)))
    return assemble(res.results)
